# revision 14
# baseline (speedup 1.0000x reference)
"""TRN2 Bass kernel: GQA attention layer (q/k/v proj + RoPE + KV-cache append +
causal flash attention + o_proj), tensor-parallel over heads across 8 NeuronCores.

Sharding: core c owns q heads 4c..4c+3 and kv head c. x is replicated
(pre-transposed on host); projection weights / caches are sharded on the head
axis; each core emits a partial o_proj output (summed on host = the all-reduce)
plus its kv-head's new k/v rows (assembled on host).

Compute dtype: float32r (f32 storage, full-rate PE path, ~2^-13 precision) with
f32 PSUM accumulation. Softmax runs without max-subtraction (scores are bounded
~±14 for this problem family) so exp/sum/divide fuse into the matmul pipeline.

Device layout notes:
 - rows are b-major: row r = b*1024 + s, r in [0, 2048)
 - q/k are produced head-dim-major ("qT": [d, rows]) so RoPE pairs land in
   partition blocks: host permutes weight rows so partitions 0:64 hold even
   (real) pair members and 64:128 hold odd members of each head.
 - scores are computed transposed [t, s] per (b, h); exp'd tiles feed both the
   PV matmul (lhsT = v rows) and a ones-matmul that forms the softmax sums.
"""
import sys

if "/opt/trn_rl_repo" not in sys.path:
    sys.path.insert(0, "/opt/trn_rl_repo")

import numpy as np

import concourse.bacc as bacc
import concourse.mybir as mybir
import concourse.tile as tile
from concourse.bass_utils import run_bass_kernel_spmd

B, S, HID = 2, 1024, 4096
NQ, NG, D = 32, 4, 128
NKV = NQ // NG
OFF = 1024
T = OFF + S
N_CORES = 8
HQ = NQ // N_CORES          # q heads per core = 4
QD = HQ * D                 # per-core q dims = 512
ROWS = B * S                # 2048
NK = HID // 128             # 32 contraction chunks
F32, F32R = mybir.dt.float32, mybir.dt.float32r
NEG = -1e30


def build_nc(reps: int = 1, phases: str = "PAO"):
    nc = bacc.Bacc(None, target_bir_lowering=False)

    xT = nc.declare_dram_parameter("xT", [HID, ROWS], F32R, isOutput=False)
    wcat = nc.declare_dram_parameter("wcat", [HID, QD + 2 * D], F32R, isOutput=False)
    bcat = nc.declare_dram_parameter("bcat", [128, HQ + 2], F32, isOutput=False)
    owT = nc.declare_dram_parameter("owT", [QD, HID], F32R, isOutput=False)
    kTc = nc.declare_dram_parameter("kTc", [B, HQ, D, OFF], F32R, isOutput=False)
    vc = nc.declare_dram_parameter("vc", [B, HQ, OFF, D], F32R, isOutput=False)
    cosT = nc.declare_dram_parameter("cosT", [64, S], F32, isOutput=False)
    sinT = nc.declare_dram_parameter("sinT", [64, S], F32, isOutput=False)
    masks = nc.declare_dram_parameter("masks", [4, 128, 512], F32, isOutput=False)
    identD = nc.declare_dram_parameter("ident", [128, 128], F32R, isOutput=False)
    onesD = nc.declare_dram_parameter("ones", [128, 512], F32R, isOutput=False)

    o_part = nc.declare_dram_parameter("o_part", [ROWS, HID], F32, isOutput=True)
    k_new = nc.declare_dram_parameter("k_new", [ROWS, D], F32, isOutput=True)
    v_new = nc.declare_dram_parameter("v_new", [ROWS, D], F32, isOutput=True)

    with tile.TileContext(nc) as tc:
        with (
            tc.tile_pool(name="const", bufs=1) as cpool,
            tc.tile_pool(name="persist", bufs=1) as ppool,
        ):
            ones = cpool.tile([128, 512], F32R)
            nc.sync.dma_start(ones[:], onesD[:])
            ident = cpool.tile([128, 128], F32R)
            nc.sync.dma_start(ident[:], identD[:])
            cos_sb = cpool.tile([64, S], F32)
            sin_sb = cpool.tile([64, S], F32)
            nc.sync.dma_start(cos_sb[:], cosT[:])
            nc.sync.dma_start(sin_sb[:], sinT[:])
            mask_sb = cpool.tile([128, 4, 512], F32)
            nc.sync.dma_start(mask_sb[:], masks.ap().rearrange("m p s -> p m s"))
            bcat_sb = cpool.tile([128, HQ + 2], F32)
            nc.sync.dma_start(bcat_sb[:], bcat[:])

            # persistent across phases
            qT_sb = ppool.tile([128, HQ, ROWS], F32R)    # rope'd q, dim-major
            kTn_sb = ppool.tile([128, ROWS], F32R)       # rope'd new k, dim-major
            vrow_sb = ppool.tile([128, ROWS // 128, D], F32R)  # new v, row-major
            attT_sb = ppool.tile([128, HQ, ROWS], F32R)  # attention out, dim-major
            owT0_sb = ppool.tile([128, HQ, HID // 2], F32R)  # o_w.T first half, prefetched
            nc.sync.dma_start(
                owT0_sb[:], owT.ap()[:, 0:HID // 2].rearrange("(h p) n -> p h n", p=128))

            def body(_iv):
                from contextlib import ExitStack

                with ExitStack() as octx:
                  # ---------------- Phase P: projections + rope ----------------
                  if "P" not in phases:
                    return
                  with ExitStack() as ctx:
                    xpool = ctx.enter_context(tc.tile_pool(name="xp", bufs=4))
                    wpool = ctx.enter_context(tc.tile_pool(name="wp", bufs=3))
                    tpool = ctx.enter_context(tc.tile_pool(name="tp", bufs=6))
                    stg = ctx.enter_context(tc.tile_pool(name="stg", bufs=3))
                    pq = ctx.enter_context(tc.tile_pool(name="pq", bufs=1, space="PSUM"))
                    pk = ctx.enter_context(tc.tile_pool(name="pk", bufs=1, space="PSUM"))
                    pv = ctx.enter_context(tc.tile_pool(name="pv", bufs=1, space="PSUM"))

                    vT_sb = stg.tile([128, ROWS], F32R, name="vT_sb", bufs=1)

                    for a in range(4):
                        pos = (a % 2) * 512
                        qp = [pq.tile([128, 512], F32, name=f"qp{m}") for m in range(HQ)]
                        kp = pk.tile([128, 512], F32, name="kp")
                        vp = pv.tile([128, 512], F32, name="vp")
                        for kg in range(NK // 2):
                            xk = xpool.tile([128, 2, 512], F32R, name="xk")
                            nc.sync.dma_start(
                                xk[:],
                                xT[256 * kg:256 * (kg + 1),
                                   512 * a:512 * (a + 1)].rearrange("(c p) s -> p c s", p=128))
                            wk = wpool.tile([128, 2, QD + 2 * D], F32R, name="wk")
                            nc.sync.dma_start(
                                wk[:],
                                wcat[256 * kg:256 * (kg + 1), :].rearrange("(c p) n -> p c n", p=128))
                            for c2 in range(2):
                                k = 2 * kg + c2
                                first, last = k == 0, k == NK - 1
                                for m in range(HQ):
                                    nc.tensor.matmul(qp[m][:], wk[:, c2, 128 * m:128 * (m + 1)],
                                                     xk[:, c2, :], start=first, stop=last)
                                nc.tensor.matmul(kp[:], wk[:, c2, QD:QD + D], xk[:, c2, :],
                                                 start=first, stop=last)
                                nc.tensor.matmul(vp[:], wk[:, c2, QD + D:], xk[:, c2, :],
                                                 start=first, stop=last)
                        for m in range(HQ):
                            nc.scalar.activation(qp[m][:], qp[m][:],
                                                 mybir.ActivationFunctionType.Identity,
                                                 bias=bcat_sb[:, m:m + 1])
                        nc.scalar.activation(kp[:], kp[:], mybir.ActivationFunctionType.Identity,
                                             bias=bcat_sb[:, HQ:HQ + 1])
                        nc.scalar.activation(vp[:], vp[:], mybir.ActivationFunctionType.Identity,
                                             bias=bcat_sb[:, HQ + 1:HQ + 2])

                        cs = cos_sb[:, pos:pos + 512]
                        sn = sin_sb[:, pos:pos + 512]
                        cols = slice(512 * a, 512 * (a + 1))
                        for m in range(HQ):
                            E, O = qp[m][0:64, :], qp[m][64:128, :]
                            t1 = tpool.tile([64, 512], F32, name="t1", tag="rt")
                            t2 = tpool.tile([64, 512], F32, name="t2", tag="rt")
                            nc.vector.tensor_mul(t1[:], E, cs)
                            nc.vector.tensor_mul(t2[:], O, sn)
                            nc.vector.tensor_sub(qT_sb[0:64, m, cols], t1[:], t2[:])
                            t3 = tpool.tile([64, 512], F32, name="t3", tag="rt")
                            t4 = tpool.tile([64, 512], F32, name="t4", tag="rt")
                            nc.vector.tensor_mul(t3[:], E, sn)
                            nc.vector.tensor_mul(t4[:], O, cs)
                            nc.vector.tensor_add(qT_sb[64:128, m, cols], t3[:], t4[:])
                        E, O = kp[0:64, :], kp[64:128, :]
                        t1 = tpool.tile([64, 512], F32, name="kt1", tag="rt")
                        t2 = tpool.tile([64, 512], F32, name="kt2", tag="rt")
                        nc.vector.tensor_mul(t1[:], E, cs)
                        nc.vector.tensor_mul(t2[:], O, sn)
                        nc.vector.tensor_sub(kTn_sb[0:64, cols], t1[:], t2[:])
                        t3 = tpool.tile([64, 512], F32, name="kt3", tag="rt")
                        t4 = tpool.tile([64, 512], F32, name="kt4", tag="rt")
                        nc.vector.tensor_mul(t3[:], E, sn)
                        nc.vector.tensor_mul(t4[:], O, cs)
                        nc.vector.tensor_add(kTn_sb[64:128, cols], t3[:], t4[:])
                        nc.scalar.copy(vT_sb[:, cols], vp[:])

                    # transpose v to row-major + k_new/v_new outputs
                    ptp = ctx.enter_context(tc.tile_pool(name="ptp", bufs=1, space="PSUM"))
                    for t in range(ROWS // 128):
                        tp1 = ptp.tile([128, 128], F32R, name="tp1")
                        nc.tensor.transpose(tp1[:], vT_sb[:, 128 * t:128 * (t + 1)], ident[:])
                        nc.vector.tensor_copy(vrow_sb[:, t, :], tp1[:])
                        nc.sync.dma_start(v_new[128 * t:128 * (t + 1), :],
                                          vrow_sb[:, t, :].bitcast(F32))
                        tp2 = ptp.tile([128, 128], F32R, name="tp2")
                        nc.tensor.transpose(tp2[:], kTn_sb[:, 128 * t:128 * (t + 1)], ident[:])
                        ks = stg.tile([128, 128], F32, name="ks")
                        nc.scalar.copy(ks[:], tp2[:])
                        nc.sync.dma_start(k_new[128 * t:128 * (t + 1), :], ks[:])

                  # ---------------- Phase A: attention ----------------
                  if "A" not in phases:
                    return
                  with ExitStack() as ctx:
                    apool = ctx.enter_context(tc.tile_pool(name="ap", bufs=2))
                    expool = ctx.enter_context(tc.tile_pool(name="ex", bufs=4))
                    rpool = ctx.enter_context(tc.tile_pool(name="rp", bufs=2))
                    ps = ctx.enter_context(tc.tile_pool(name="ps", bufs=3, space="PSUM"))
                    po = ctx.enter_context(tc.tile_pool(name="po", bufs=2, space="PSUM"))
                    pu = ctx.enter_context(tc.tile_pool(name="pu", bufs=1, space="PSUM"))
                    pb = ctx.enter_context(tc.tile_pool(name="pb", bufs=1, space="PSUM"))

                    for b in range(B):
                        for h in range(HQ):
                            kc_sb = apool.tile([128, OFF // 128, 128], F32R, name="kc_sb")
                            vc_sb = apool.tile([128, OFF // 128, 128], F32R, name="vc_sb")
                            nc.sync.dma_start(
                                kc_sb[:], kTc.ap()[b, h].rearrange("d (j t) -> d j t", t=128))
                            nc.sync.dma_start(
                                vc_sb[:], vc.ap()[b, h].rearrange("(j p) d -> p j d", p=128))
                            for a2 in range(2):
                                rq = qT_sb[:, h, b * S + 512 * a2: b * S + 512 * (a2 + 1)]
                                oup = po.tile([128, 512], F32, name="oup")
                                sup = pu.tile([1, 512], F32, name="sup")
                                n_new = 4 * a2 + 4
                                n_tot = 8 + n_new
                                for idx in range(n_tot):
                                    sp = ps.tile([128, 512], F32, name="sp")
                                    if idx < 8:
                                        lh = kc_sb[:, idx, :]
                                        vt = vc_sb[:, idx, :]
                                    else:
                                        j = idx - 8
                                        lh = kTn_sb[:, b * S + 128 * j: b * S + 128 * (j + 1)]
                                        vt = vrow_sb[:, 8 * b + j, :]
                                    nc.tensor.matmul(sp[:], lh, rq, start=True, stop=True)
                                    if idx >= 8 and (j - 4 * a2) >= 0:
                                        nc.vector.tensor_add(
                                            sp[:], sp[:], mask_sb[:, j - 4 * a2, :])
                                    ex = expool.tile([128, 512], F32R, name="ex")
                                    nc.scalar.activation(
                                        ex[:], sp[:], mybir.ActivationFunctionType.Exp)
                                    st, sp_ = idx == 0, idx == n_tot - 1
                                    nc.tensor.matmul(sup[:], ones[:, 0:1], ex[:],
                                                     start=st, stop=sp_)
                                    nc.tensor.matmul(oup[:], vt, ex[:], start=st, stop=sp_)
                                rec = rpool.tile([1, 512], F32R, name="rec")
                                with nc.allow_low_precision(reason="softmax recip row, f32r for PE broadcast"):
                                    nc.vector.reciprocal(rec[:], sup[:])
                                bcp = pb.tile([128, 512], F32, name="bcp")
                                nc.tensor.matmul(bcp[:], ones[0:1, 0:128], rec[:],
                                                 start=True, stop=True)
                                bcs = rpool.tile([128, 512], F32, name="bcs")
                                nc.vector.tensor_copy(bcs[:], bcp[:])
                                nc.vector.tensor_mul(
                                    attT_sb[:, h, b * S + 512 * a2: b * S + 512 * (a2 + 1)],
                                    oup[:], bcs[:])

                # ---------------- Phase O: output projection ----------------
                if "O" not in phases:
                    return
                with ExitStack() as ctx:
                    ostg = ctx.enter_context(tc.tile_pool(name="ostg", bufs=4))
                    owp = ctx.enter_context(tc.tile_pool(name="owp", bufs=1))
                    pop = ctx.enter_context(tc.tile_pool(name="pop", bufs=4, space="PSUM"))
                    owT1_sb = owp.tile([128, HQ, HID // 2], F32R, name="owT1_sb")
                    nc.sync.dma_start(
                        owT1_sb[:],
                        owT.ap()[:, HID // 2:].rearrange("(h p) n -> p h n", p=128))
                    for half, wsb in ((0, owT0_sb), (1, owT1_sb)):
                        for st in range(ROWS // 128):
                            for oc in range(HID // 1024):
                                op = pop.tile([128, 512], F32, name="op")
                                for hk in range(HQ):
                                    nc.tensor.matmul(
                                        op[:], attT_sb[:, hk, 128 * st:128 * (st + 1)],
                                        wsb[:, hk, 512 * oc:512 * (oc + 1)],
                                        start=(hk == 0), stop=(hk == HQ - 1))
                                ost = ostg.tile([128, 512], F32, name="ost")
                                if oc % 2 == 0:
                                    nc.scalar.copy(ost[:], op[:])
                                else:
                                    nc.vector.tensor_copy(ost[:], op[:])
                                nc.sync.dma_start(
                                    o_part[128 * st:128 * (st + 1),
                                           half * (HID // 2) + 512 * oc:
                                           half * (HID // 2) + 512 * (oc + 1)],
                                    ost[:])

            if reps > 1:
                with tc.For_i(0, reps, 1) as iv:
                    body(iv)
            else:
                body(None)

    nc.compile()
    return nc


_PERM = np.concatenate([np.arange(0, D, 2), np.arange(1, D, 2)])   # evens then odds
_INV_PERM = np.argsort(_PERM)


def _host_prep(inputs):
    """Shard + lay out all inputs per core. Returns in_maps list."""
    f32 = np.float32
    x = np.asarray(inputs["x"], f32)
    qw = np.asarray(inputs["q_w"], f32)
    qb_ = np.asarray(inputs["q_b"], f32)
    kw = np.asarray(inputs["k_w"], f32)
    kb_ = np.asarray(inputs["k_b"], f32)
    vw = np.asarray(inputs["v_w"], f32)
    vb_ = np.asarray(inputs["v_b"], f32)
    ow = np.asarray(inputs["o_w"], f32)
    kc = np.asarray(inputs["k_cache"], f32)
    vcache = np.asarray(inputs["v_cache"], f32)
    cos = np.asarray(inputs["precomp_freqs_cos"], f32)
    sin = np.asarray(inputs["precomp_freqs_sin"], f32)

    scale = f32(1.0 / np.sqrt(D))
    xT = np.ascontiguousarray(x.reshape(ROWS, HID).T)
    cosT = np.ascontiguousarray(cos.T)
    sinT = np.ascontiguousarray(sin.T)

    t_rel = np.arange(128)[:, None]
    s_rel = np.arange(512)[None, :]
    masks = np.stack(
        [np.where(128 * p + t_rel > s_rel, f32(NEG), f32(0.0)) for p in range(4)]
    ).astype(f32)

    in_maps = []
    for c in range(N_CORES):
        qws = (qw[QD * c:QD * (c + 1)] * scale).reshape(HQ, D, HID)[:, _PERM].reshape(QD, HID)
        qbs = (qb_[QD * c:QD * (c + 1)] * scale).reshape(HQ, D)[:, _PERM].reshape(1, QD)
        kws = kw[D * c:D * (c + 1)][_PERM]
        kbs = kb_[D * c:D * (c + 1)][_PERM].reshape(1, D)
        vws = vw[D * c:D * (c + 1)]
        vbs = vb_[D * c:D * (c + 1)].reshape(1, D)
        kcs = kc[:, :, HQ * c:HQ * (c + 1), :][..., _PERM]          # [B, OFF, 4, D]
        in_maps.append({
            "xT": xT,
            "wcat": np.ascontiguousarray(
                np.concatenate([qws.T, kws.T, vws.T], axis=1)),
            "bcat": np.ascontiguousarray(np.concatenate(
                [qbs.reshape(HQ, D).T, kbs.reshape(1, D).T, vbs.reshape(1, D).T],
                axis=1)),
            "owT": np.ascontiguousarray(ow[:, QD * c:QD * (c + 1)].T),
            "kTc": np.ascontiguousarray(kcs.transpose(0, 2, 3, 1)),  # [B, 4, D, OFF]
            "vc": np.ascontiguousarray(
                vcache[:, :, HQ * c:HQ * (c + 1), :].transpose(0, 2, 1, 3)),
            "ident": np.eye(128, dtype=f32),
            "ones": np.ones((128, 512), f32),
            "cosT": cosT,
            "sinT": sinT,
            "masks": masks,
        })
    return in_maps


def _assemble(inputs, results):
    f32 = np.float32
    kc = np.asarray(inputs["k_cache"], f32)
    vcache = np.asarray(inputs["v_cache"], f32)
    o = np.zeros((ROWS, HID), f32)
    for c in range(N_CORES):
        o += results[c]["o_part"]
    k_full = np.empty((B, T, NQ, D), f32)
    v_full = np.empty((B, T, NQ, D), f32)
    k_full[:, :OFF] = kc
    v_full[:, :OFF] = vcache
    for c in range(N_CORES):
        kn = results[c]["k_new"].reshape(B, S, D)[:, :, _INV_PERM]
        vn = results[c]["v_new"].reshape(B, S, D)
        for h in range(HQ * c, HQ * (c + 1)):
            k_full[:, OFF:, h, :] = kn
            v_full[:, OFF:, h, :] = vn
    return o.reshape(B, S, HID), k_full, v_full


def kernel(**inputs):
    in_maps = _host_prep(inputs)
    nc = build_nc(reps=1)
    res = run_bass_kernel_spmd(nc, in_maps, list(range(N_CORES)))
    return _assemble(inputs, res.results)


# revision 15
# speedup vs baseline: 1.1245x; 1.1245x over previous
"""TRN2 Bass kernel: GQA attention layer (q/k/v proj + RoPE + KV-cache append +
causal flash attention + o_proj), tensor-parallel over heads across 8 NeuronCores.

Sharding: core c owns q heads 4c..4c+3 and kv head c. x is replicated
(pre-transposed on host); projection weights / caches are sharded on the head
axis; each core emits a partial o_proj output (summed on host = the all-reduce)
plus its kv-head's new k/v rows (assembled on host).

Compute dtype: float32r (f32 storage, full-rate PE path, ~2^-13 precision) with
f32 PSUM accumulation. Softmax runs without max-subtraction (scores are bounded
~±14 for this problem family) so exp/sum/divide fuse into the matmul pipeline.

Device layout notes:
 - rows are b-major: row r = b*1024 + s, r in [0, 2048)
 - q/k are produced head-dim-major ("qT": [d, rows]) so RoPE pairs land in
   partition blocks: host permutes weight rows so partitions 0:64 hold even
   (real) pair members and 64:128 hold odd members of each head.
 - scores are computed transposed [t, s] per (b, h); exp'd tiles feed both the
   PV matmul (lhsT = v rows) and a ones-matmul that forms the softmax sums.
"""
import sys

if "/opt/trn_rl_repo" not in sys.path:
    sys.path.insert(0, "/opt/trn_rl_repo")

import numpy as np

import concourse.bacc as bacc
import concourse.mybir as mybir
import concourse.tile as tile
from concourse.bass_utils import run_bass_kernel_spmd

B, S, HID = 2, 1024, 4096
NQ, NG, D = 32, 4, 128
NKV = NQ // NG
OFF = 1024
T = OFF + S
N_CORES = 8
HQ = NQ // N_CORES          # q heads per core = 4
QD = HQ * D                 # per-core q dims = 512
ROWS = B * S                # 2048
NK = HID // 128             # 32 contraction chunks
F32, F32R = mybir.dt.float32, mybir.dt.float32r
NEG = -1e30


def build_nc(reps: int = 1, phases: str = "PAO"):
    nc = bacc.Bacc(None, target_bir_lowering=False)

    xT = nc.declare_dram_parameter("xT", [HID, ROWS], F32R, isOutput=False)
    wcat = nc.declare_dram_parameter("wcat", [HID, QD + 2 * D], F32R, isOutput=False)
    brow = nc.declare_dram_parameter("brow", [1, QD + 2 * D], F32R, isOutput=False)
    owT = nc.declare_dram_parameter("owT", [QD, HID], F32R, isOutput=False)
    kTc = nc.declare_dram_parameter("kTc", [B, HQ, D, OFF], F32R, isOutput=False)
    vc = nc.declare_dram_parameter("vc", [B, HQ, OFF, D], F32R, isOutput=False)
    cosT = nc.declare_dram_parameter("cosT", [64, S], F32, isOutput=False)
    sinT = nc.declare_dram_parameter("sinT", [64, S], F32, isOutput=False)
    masks = nc.declare_dram_parameter("masks", [4, 128, 512], F32, isOutput=False)
    identD = nc.declare_dram_parameter("ident", [128, 128], F32R, isOutput=False)
    onesD = nc.declare_dram_parameter("ones", [128, 512], F32R, isOutput=False)

    o_part = nc.declare_dram_parameter("o_part", [ROWS, HID], F32, isOutput=True)
    k_new = nc.declare_dram_parameter("k_new", [ROWS, D], F32, isOutput=True)
    v_new = nc.declare_dram_parameter("v_new", [ROWS, D], F32, isOutput=True)

    with tile.TileContext(nc) as tc:
        with (
            tc.tile_pool(name="const", bufs=1) as cpool,
            tc.tile_pool(name="persist", bufs=1) as ppool,
        ):
            ones = cpool.tile([128, 512], F32R)
            nc.sync.dma_start(ones[:], onesD[:])
            ident = cpool.tile([128, 128], F32R)
            nc.sync.dma_start(ident[:], identD[:])
            cos_sb = cpool.tile([64, S], F32)
            sin_sb = cpool.tile([64, S], F32)
            nc.sync.dma_start(cos_sb[:], cosT[:])
            nc.sync.dma_start(sin_sb[:], sinT[:])
            mask_sb = cpool.tile([128, 4, 512], F32)
            nc.sync.dma_start(mask_sb[:], masks.ap().rearrange("m p s -> p m s"))
            brow_sb = cpool.tile([1, QD + 2 * D], F32R)
            nc.sync.dma_start(brow_sb[:], brow[:])

            # persistent across phases
            qT_sb = ppool.tile([128, HQ, ROWS], F32R)    # rope'd q, dim-major
            kTn_sb = ppool.tile([128, ROWS], F32R)       # rope'd new k, dim-major
            vrow_sb = ppool.tile([128, ROWS // 128, D], F32R)  # new v, row-major
            attT_sb = ppool.tile([128, HQ, ROWS], F32R)  # attention out, dim-major
            owT0_sb = ppool.tile([128, HQ, HID // 2], F32R)  # o_w.T first half, prefetched
            nc.sync.dma_start(
                owT0_sb[:], owT.ap()[:, 0:HID // 2].rearrange("(h p) n -> p h n", p=128))

            def body(_iv):
                from contextlib import ExitStack

                with ExitStack() as octx:
                  # ---------------- Phase P: projections + rope ----------------
                  if "P" not in phases:
                    return
                  with ExitStack() as ctx:
                    xpool = ctx.enter_context(tc.tile_pool(name="xp", bufs=4))
                    wpool = ctx.enter_context(tc.tile_pool(name="wp", bufs=3))
                    tpool = ctx.enter_context(tc.tile_pool(name="tp", bufs=6))
                    stg = ctx.enter_context(tc.tile_pool(name="stg", bufs=3))
                    pq = ctx.enter_context(tc.tile_pool(name="pq", bufs=1, space="PSUM"))
                    pk = ctx.enter_context(tc.tile_pool(name="pk", bufs=1, space="PSUM"))
                    pv = ctx.enter_context(tc.tile_pool(name="pv", bufs=1, space="PSUM"))

                    vT_sb = stg.tile([128, ROWS], F32R, name="vT_sb", bufs=1)

                    for a in range(4):
                        pos = (a % 2) * 512
                        qp = [pq.tile([128, 512], F32, name=f"qp{m}") for m in range(HQ)]
                        kp = pk.tile([128, 512], F32, name="kp")
                        vp = pv.tile([128, 512], F32, name="vp")
                        for m in range(HQ):
                            nc.tensor.matmul(qp[m][:], brow_sb[:, 128 * m:128 * (m + 1)],
                                             ones[0:1, :], start=True, stop=False)
                        nc.tensor.matmul(kp[:], brow_sb[:, QD:QD + D], ones[0:1, :],
                                         start=True, stop=False)
                        nc.tensor.matmul(vp[:], brow_sb[:, QD + D:], ones[0:1, :],
                                         start=True, stop=False)
                        for kg in range(NK // 2):
                            xk = xpool.tile([128, 2, 512], F32R, name="xk")
                            nc.sync.dma_start(
                                xk[:],
                                xT[256 * kg:256 * (kg + 1),
                                   512 * a:512 * (a + 1)].rearrange("(c p) s -> p c s", p=128))
                            wk = wpool.tile([128, 2, QD + 2 * D], F32R, name="wk")
                            nc.sync.dma_start(
                                wk[:],
                                wcat[256 * kg:256 * (kg + 1), :].rearrange("(c p) n -> p c n", p=128))
                            for c2 in range(2):
                                k = 2 * kg + c2
                                last = k == NK - 1
                                for m in range(HQ):
                                    nc.tensor.matmul(qp[m][:], wk[:, c2, 128 * m:128 * (m + 1)],
                                                     xk[:, c2, :], start=False, stop=last)
                                nc.tensor.matmul(kp[:], wk[:, c2, QD:QD + D], xk[:, c2, :],
                                                 start=False, stop=last)
                                nc.tensor.matmul(vp[:], wk[:, c2, QD + D:], xk[:, c2, :],
                                                 start=False, stop=last)

                        cs = cos_sb[:, pos:pos + 512]
                        sn = sin_sb[:, pos:pos + 512]
                        cols = slice(512 * a, 512 * (a + 1))
                        for m in range(HQ):
                            E, O = qp[m][0:64, :], qp[m][64:128, :]
                            t1 = tpool.tile([64, 512], F32, name="t1", tag="rt")
                            t2 = tpool.tile([64, 512], F32, name="t2", tag="rt")
                            nc.vector.tensor_mul(t1[:], E, cs)
                            nc.vector.tensor_mul(t2[:], O, sn)
                            nc.vector.tensor_sub(qT_sb[0:64, m, cols], t1[:], t2[:])
                            t3 = tpool.tile([64, 512], F32, name="t3", tag="rt")
                            t4 = tpool.tile([64, 512], F32, name="t4", tag="rt")
                            nc.vector.tensor_mul(t3[:], E, sn)
                            nc.vector.tensor_mul(t4[:], O, cs)
                            nc.vector.tensor_add(qT_sb[64:128, m, cols], t3[:], t4[:])
                        E, O = kp[0:64, :], kp[64:128, :]
                        t1 = tpool.tile([64, 512], F32, name="kt1", tag="rt")
                        t2 = tpool.tile([64, 512], F32, name="kt2", tag="rt")
                        nc.vector.tensor_mul(t1[:], E, cs)
                        nc.vector.tensor_mul(t2[:], O, sn)
                        nc.vector.tensor_sub(kTn_sb[0:64, cols], t1[:], t2[:])
                        t3 = tpool.tile([64, 512], F32, name="kt3", tag="rt")
                        t4 = tpool.tile([64, 512], F32, name="kt4", tag="rt")
                        nc.vector.tensor_mul(t3[:], E, sn)
                        nc.vector.tensor_mul(t4[:], O, cs)
                        nc.vector.tensor_add(kTn_sb[64:128, cols], t3[:], t4[:])
                        nc.scalar.copy(vT_sb[:, cols], vp[:])

                    # transpose v to row-major + k_new/v_new outputs
                    ptp = ctx.enter_context(tc.tile_pool(name="ptp", bufs=1, space="PSUM"))
                    for t in range(ROWS // 128):
                        tp1 = ptp.tile([128, 128], F32R, name="tp1")
                        nc.tensor.transpose(tp1[:], vT_sb[:, 128 * t:128 * (t + 1)], ident[:])
                        nc.vector.tensor_copy(vrow_sb[:, t, :], tp1[:])
                        nc.sync.dma_start(v_new[128 * t:128 * (t + 1), :],
                                          vrow_sb[:, t, :].bitcast(F32))
                        tp2 = ptp.tile([128, 128], F32R, name="tp2")
                        nc.tensor.transpose(tp2[:], kTn_sb[:, 128 * t:128 * (t + 1)], ident[:])
                        ks = stg.tile([128, 128], F32, name="ks")
                        nc.scalar.copy(ks[:], tp2[:])
                        nc.sync.dma_start(k_new[128 * t:128 * (t + 1), :], ks[:])

                  # ---------------- Phase A: attention ----------------
                  if "A" not in phases:
                    return
                  with ExitStack() as ctx:
                    apool = ctx.enter_context(tc.tile_pool(name="ap", bufs=2))
                    expool = ctx.enter_context(tc.tile_pool(name="ex", bufs=4))
                    rpool = ctx.enter_context(tc.tile_pool(name="rp", bufs=2))
                    ps = ctx.enter_context(tc.tile_pool(name="ps", bufs=3, space="PSUM"))
                    po = ctx.enter_context(tc.tile_pool(name="po", bufs=2, space="PSUM"))
                    pu = ctx.enter_context(tc.tile_pool(name="pu", bufs=1, space="PSUM"))
                    pb = ctx.enter_context(tc.tile_pool(name="pb", bufs=1, space="PSUM"))

                    for b in range(B):
                        for h in range(HQ):
                            kc_sb = apool.tile([128, OFF // 128, 128], F32R, name="kc_sb")
                            vc_sb = apool.tile([128, OFF // 128, 128], F32R, name="vc_sb")
                            nc.sync.dma_start(
                                kc_sb[:], kTc.ap()[b, h].rearrange("d (j t) -> d j t", t=128))
                            nc.sync.dma_start(
                                vc_sb[:], vc.ap()[b, h].rearrange("(j p) d -> p j d", p=128))
                            for a2 in range(2):
                                rq = qT_sb[:, h, b * S + 512 * a2: b * S + 512 * (a2 + 1)]
                                oup = po.tile([128, 512], F32, name="oup")
                                sup = pu.tile([1, 512], F32, name="sup")
                                n_new = 4 * a2 + 4
                                n_tot = 8 + n_new
                                for idx in range(n_tot):
                                    sp = ps.tile([128, 512], F32, name="sp")
                                    if idx < 8:
                                        lh = kc_sb[:, idx, :]
                                        vt = vc_sb[:, idx, :]
                                    else:
                                        j = idx - 8
                                        lh = kTn_sb[:, b * S + 128 * j: b * S + 128 * (j + 1)]
                                        vt = vrow_sb[:, 8 * b + j, :]
                                    nc.tensor.matmul(sp[:], lh, rq, start=True, stop=True)
                                    if idx >= 8 and (j - 4 * a2) >= 0:
                                        nc.vector.tensor_add(
                                            sp[:], sp[:], mask_sb[:, j - 4 * a2, :])
                                    ex = expool.tile([128, 512], F32R, name="ex")
                                    nc.scalar.activation(
                                        ex[:], sp[:], mybir.ActivationFunctionType.Exp)
                                    st, sp_ = idx == 0, idx == n_tot - 1
                                    nc.tensor.matmul(sup[:], ones[:, 0:1], ex[:],
                                                     start=st, stop=sp_)
                                    nc.tensor.matmul(oup[:], vt, ex[:], start=st, stop=sp_)
                                rec = rpool.tile([1, 512], F32R, name="rec")
                                with nc.allow_low_precision(reason="softmax recip row, f32r for PE broadcast"):
                                    nc.vector.reciprocal(rec[:], sup[:])
                                bcp = pb.tile([128, 512], F32, name="bcp")
                                nc.tensor.matmul(bcp[:], ones[0:1, 0:128], rec[:],
                                                 start=True, stop=True)
                                bcs = rpool.tile([128, 512], F32, name="bcs")
                                nc.vector.tensor_copy(bcs[:], bcp[:])
                                nc.vector.tensor_mul(
                                    attT_sb[:, h, b * S + 512 * a2: b * S + 512 * (a2 + 1)],
                                    oup[:], bcs[:])

                # ---------------- Phase O: output projection ----------------
                if "O" not in phases:
                    return
                with ExitStack() as ctx:
                    ostg = ctx.enter_context(tc.tile_pool(name="ostg", bufs=4))
                    owp = ctx.enter_context(tc.tile_pool(name="owp", bufs=1))
                    pop = ctx.enter_context(tc.tile_pool(name="pop", bufs=4, space="PSUM"))
                    owT1_sb = owp.tile([128, HQ, HID // 2], F32R, name="owT1_sb")
                    nc.sync.dma_start(
                        owT1_sb[:],
                        owT.ap()[:, HID // 2:].rearrange("(h p) n -> p h n", p=128))
                    for half, wsb in ((0, owT0_sb), (1, owT1_sb)):
                        for st in range(ROWS // 128):
                            for oc in range(HID // 1024):
                                op = pop.tile([128, 512], F32, name="op")
                                for hk in range(HQ):
                                    nc.tensor.matmul(
                                        op[:], attT_sb[:, hk, 128 * st:128 * (st + 1)],
                                        wsb[:, hk, 512 * oc:512 * (oc + 1)],
                                        start=(hk == 0), stop=(hk == HQ - 1))
                                ost = ostg.tile([128, 512], F32, name="ost")
                                if oc % 2 == 0:
                                    nc.scalar.copy(ost[:], op[:])
                                else:
                                    nc.vector.tensor_copy(ost[:], op[:])
                                nc.sync.dma_start(
                                    o_part[128 * st:128 * (st + 1),
                                           half * (HID // 2) + 512 * oc:
                                           half * (HID // 2) + 512 * (oc + 1)],
                                    ost[:])

            if reps > 1:
                with tc.For_i(0, reps, 1) as iv:
                    body(iv)
            else:
                body(None)

    nc.compile()
    return nc


_PERM = np.concatenate([np.arange(0, D, 2), np.arange(1, D, 2)])   # evens then odds
_INV_PERM = np.argsort(_PERM)


def _host_prep(inputs):
    """Shard + lay out all inputs per core. Returns in_maps list."""
    f32 = np.float32
    x = np.asarray(inputs["x"], f32)
    qw = np.asarray(inputs["q_w"], f32)
    qb_ = np.asarray(inputs["q_b"], f32)
    kw = np.asarray(inputs["k_w"], f32)
    kb_ = np.asarray(inputs["k_b"], f32)
    vw = np.asarray(inputs["v_w"], f32)
    vb_ = np.asarray(inputs["v_b"], f32)
    ow = np.asarray(inputs["o_w"], f32)
    kc = np.asarray(inputs["k_cache"], f32)
    vcache = np.asarray(inputs["v_cache"], f32)
    cos = np.asarray(inputs["precomp_freqs_cos"], f32)
    sin = np.asarray(inputs["precomp_freqs_sin"], f32)

    scale = f32(1.0 / np.sqrt(D))
    xT = np.ascontiguousarray(x.reshape(ROWS, HID).T)
    cosT = np.ascontiguousarray(cos.T)
    sinT = np.ascontiguousarray(sin.T)

    t_rel = np.arange(128)[:, None]
    s_rel = np.arange(512)[None, :]
    masks = np.stack(
        [np.where(128 * p + t_rel > s_rel, f32(NEG), f32(0.0)) for p in range(4)]
    ).astype(f32)

    in_maps = []
    for c in range(N_CORES):
        qws = (qw[QD * c:QD * (c + 1)] * scale).reshape(HQ, D, HID)[:, _PERM].reshape(QD, HID)
        qbs = (qb_[QD * c:QD * (c + 1)] * scale).reshape(HQ, D)[:, _PERM].reshape(1, QD)
        kws = kw[D * c:D * (c + 1)][_PERM]
        kbs = kb_[D * c:D * (c + 1)][_PERM].reshape(1, D)
        vws = vw[D * c:D * (c + 1)]
        vbs = vb_[D * c:D * (c + 1)].reshape(1, D)
        kcs = kc[:, :, HQ * c:HQ * (c + 1), :][..., _PERM]          # [B, OFF, 4, D]
        in_maps.append({
            "xT": xT,
            "wcat": np.ascontiguousarray(
                np.concatenate([qws.T, kws.T, vws.T], axis=1)),
            "brow": np.ascontiguousarray(np.concatenate(
                [qbs.ravel(), kbs.ravel(), vbs.ravel()]).reshape(1, QD + 2 * D)),
            "owT": np.ascontiguousarray(ow[:, QD * c:QD * (c + 1)].T),
            "kTc": np.ascontiguousarray(kcs.transpose(0, 2, 3, 1)),  # [B, 4, D, OFF]
            "vc": np.ascontiguousarray(
                vcache[:, :, HQ * c:HQ * (c + 1), :].transpose(0, 2, 1, 3)),
            "ident": np.eye(128, dtype=f32),
            "ones": np.ones((128, 512), f32),
            "cosT": cosT,
            "sinT": sinT,
            "masks": masks,
        })
    return in_maps


def _assemble(inputs, results):
    f32 = np.float32
    kc = np.asarray(inputs["k_cache"], f32)
    vcache = np.asarray(inputs["v_cache"], f32)
    o = np.zeros((ROWS, HID), f32)
    for c in range(N_CORES):
        o += results[c]["o_part"]
    k_full = np.empty((B, T, NQ, D), f32)
    v_full = np.empty((B, T, NQ, D), f32)
    k_full[:, :OFF] = kc
    v_full[:, :OFF] = vcache
    for c in range(N_CORES):
        kn = results[c]["k_new"].reshape(B, S, D)[:, :, _INV_PERM]
        vn = results[c]["v_new"].reshape(B, S, D)
        for h in range(HQ * c, HQ * (c + 1)):
            k_full[:, OFF:, h, :] = kn
            v_full[:, OFF:, h, :] = vn
    return o.reshape(B, S, HID), k_full, v_full


def kernel(**inputs):
    in_maps = _host_prep(inputs)
    nc = build_nc(reps=1)
    res = run_bass_kernel_spmd(nc, in_maps, list(range(N_CORES)))
    return _assemble(inputs, res.results)
